# revision 13
# baseline (speedup 1.0000x reference)
"""Trainium2 Bass kernel for AttentionNet pooling.

Computation (per batch b):
    c[b,:]   = rel[b] @ Wr^T + pool[b] @ Wg^T + Wr_b + Wg_b + Wh_b          [H]
    v[s,k]   = sent_h[b,s,:] @ Wh^T[:,k]                                     [S,H]
    w[s]     = alpha . tanh(c + v[s,:]) + alpha_b                            [S]
    e[s]     = exp(w[s]) * mask[b,s]          (exp without max-sub; masking
                                               multiplicatively after exp)
    weight_  = e / sum(e)
    pooled   = sum_s e[s] * sent_h[b,s,:] / sum(e)                           [H]
    att_res  = pooled @ Wh^T + Wh_b     (since sum(weight_) == 1)            [H]

Sharding: data-parallel over batch B=64 across 8 cores (8 batches/core),
weights replicated.  sent_h is read exactly once from HBM (32 MB/core).
Matmul operands are typed float32r (same fp32 bytes, PE streams 1 col/cycle
instead of 4 for plain fp32, TF32-class precision).
"""

import sys

if "/opt/trn_rl_repo" not in sys.path:
    sys.path.insert(0, "/opt/trn_rl_repo")

import numpy as np

B, S, H, R = 64, 2048, 512, 256
NCORES = 8
BL = B // NCORES  # batches per core
P = 128
KC = H // P      # 4 chunks of the H (contraction / output) dim
SG = S // 512    # 4 s-groups of 512
SUB = 512 // P   # 4 s-subblocks of 128 inside a group

_NC_CACHE = None


def build_nc():
    """Build (once) the single-core Bass/Tile program run SPMD on all cores."""
    import concourse.mybir as mybir
    import concourse.tile as tile
    from concourse import bacc
    from concourse.bass import ts, ds

    fp32 = mybir.dt.float32
    f32r = mybir.dt.float32r
    i32 = mybir.dt.int32
    Tanh = mybir.ActivationFunctionType.Tanh
    Exp = mybir.ActivationFunctionType.Exp
    AX = mybir.AxisListType.X
    MUL = mybir.AluOpType.mult

    nc = bacc.Bacc("TRN2", target_bir_lowering=False, debug=False)

    sent = nc.dram_tensor("sent", [BL, S, H], f32r, kind="ExternalInput")
    mask = nc.dram_tensor("mask", [BL, S], i32, kind="ExternalInput")
    relT = nc.dram_tensor("relT", [R, BL], f32r, kind="ExternalInput")
    poolT = nc.dram_tensor("poolT", [H, BL], f32r, kind="ExternalInput")
    wgT = nc.dram_tensor("wgT", [H, H], f32r, kind="ExternalInput")
    whT = nc.dram_tensor("whT", [H, H], f32r, kind="ExternalInput")
    wrT = nc.dram_tensor("wrT", [R, H], f32r, kind="ExternalInput")
    alphaT = nc.dram_tensor("alphaT", [H], f32r, kind="ExternalInput")
    wgb = nc.dram_tensor("wgb", [1, H], f32r, kind="ExternalInput")
    whb = nc.dram_tensor("whb", [1, H], f32r, kind="ExternalInput")
    wrb = nc.dram_tensor("wrb", [1, H], f32r, kind="ExternalInput")
    ab = nc.dram_tensor("ab", [1, 1], fp32, kind="ExternalInput")
    idin = nc.dram_tensor("idin", [P, P], f32r, kind="ExternalInput")
    onein = nc.dram_tensor("onein", [1, P], f32r, kind="ExternalInput")
    att = nc.dram_tensor("att", [BL, H], fp32, kind="ExternalOutput")
    wout = nc.dram_tensor("wout", [BL, S], fp32, kind="ExternalOutput")

    with tile.TileContext(nc) as tc:
        with (
            tc.tile_pool(name="const", bufs=1) as constp,
            tc.tile_pool(name="u", bufs=8) as up,
            tc.tile_pool(name="uT", bufs=4) as utp,
            tc.tile_pool(name="mix", bufs=6) as mixp,
            tc.tile_pool(name="row", bufs=2) as rowp,
            tc.tile_pool(name="small", bufs=2) as smallp,
            tc.tile_pool(name="ps_tr", bufs=2, space="PSUM") as ps_tr,
            tc.tile_pool(name="ps_vt", bufs=2, space="PSUM") as ps_vt,
            tc.tile_pool(name="ps_sm", bufs=2, space="PSUM") as ps_sm,
            tc.tile_pool(name="ps_pool", bufs=2, space="PSUM") as ps_pool,
        ):
            # ---------------- constants / weights ----------------
            ident = constp.tile([P, P], f32r, tag="ident")
            nc.sync.dma_start(ident, idin[:])
            ones_r = constp.tile([1, P], f32r, tag="ones_r")
            nc.sync.dma_start(ones_r, onein[:])
            ones_f = constp.tile([1, 1], fp32, tag="ones_f")
            nc.vector.memset(ones_f, 1.0)

            whT_sb = constp.tile([P, KC, H], f32r, tag="whT")
            nc.sync.dma_start(whT_sb, whT[:].rearrange("(o p) k -> p o k", p=P))
            wgT_sb = constp.tile([P, KC, H], f32r, tag="wgT")
            nc.sync.dma_start(wgT_sb, wgT[:].rearrange("(o p) k -> p o k", p=P))
            wrT_sb = constp.tile([P, R // P, H], f32r, tag="wrT")
            nc.sync.dma_start(wrT_sb, wrT[:].rearrange("(o p) k -> p o k", p=P))
            alphaT_sb = constp.tile([P, KC], f32r, tag="alphaT")
            nc.sync.dma_start(alphaT_sb, alphaT[:].rearrange("(o p) -> p o", p=P))
            relT_sb = constp.tile([P, R // P, BL], f32r, tag="relT")
            nc.sync.dma_start(relT_sb, relT[:].rearrange("(o p) b -> p o b", p=P))
            poolT_sb = constp.tile([P, KC, BL], f32r, tag="poolT")
            nc.sync.dma_start(poolT_sb, poolT[:].rearrange("(o p) b -> p o b", p=P))
            whb_sb = constp.tile([1, H], f32r, tag="whb")
            nc.sync.dma_start(whb_sb, whb[:])
            wgb_sb = constp.tile([1, H], f32r, tag="wgb")
            nc.sync.dma_start(wgb_sb, wgb[:])
            wrb_sb = constp.tile([1, H], f32r, tag="wrb")
            nc.sync.dma_start(wrb_sb, wrb[:])
            ab_sb = constp.tile([1, 1], fp32, tag="ab")
            nc.sync.dma_start(ab_sb, ab[:])

            # ---------------- c = relation + global_sen + biases ----------------
            c_ps = ps_sm.tile([BL, H], fp32, tag="sm")
            nc.tensor.matmul(c_ps, lhsT=relT_sb[:, 0, :], rhs=wrT_sb[:, 0, :],
                             start=True, stop=False)
            nc.tensor.matmul(c_ps, lhsT=relT_sb[:, 1, :], rhs=wrT_sb[:, 1, :],
                             start=False, stop=False)
            for i in range(KC):
                nc.tensor.matmul(c_ps, lhsT=poolT_sb[:, i, :], rhs=wgT_sb[:, i, :],
                                 start=False, stop=False)
            nc.tensor.matmul(c_ps, lhsT=ones_r[0:1, :BL], rhs=wrb_sb,
                             start=False, stop=False)
            nc.tensor.matmul(c_ps, lhsT=ones_r[0:1, :BL], rhs=wgb_sb,
                             start=False, stop=False)
            nc.tensor.matmul(c_ps, lhsT=ones_r[0:1, :BL], rhs=whb_sb,
                             start=False, stop=True)
            c_sb = smallp.tile([BL, H], f32r, tag="c")
            nc.scalar.copy(c_sb, c_ps)

            # cT[:, i*BL + b] = c[b, i*128 : (i+1)*128]
            cT_ps = ps_sm.tile([P, KC * BL], fp32, tag="sm")
            for i in range(KC):
                nc.tensor.matmul(cT_ps[:, ts(i, BL)], lhsT=c_sb[:, ts(i, P)],
                                 rhs=ident[0:BL, 0:BL], start=True, stop=True)
            cT_sb = constp.tile([P, KC * BL], fp32, tag="cT")
            nc.scalar.copy(cT_sb, cT_ps)

            # ---------------- per-batch pipeline ----------------
            for b in range(BL):
                # mask row (int32 -> fp32 cast during DMA, SWDGE)
                mrow = rowp.tile([1, S], fp32, tag="mrow")
                nc.gpsimd.dma_start(mrow, mask[b : b + 1, :])

                # load sent_h[b] as 4 x [128s, 4sub, 512h]
                u_tiles = []
                for g in range(SG):
                    u_t = up.tile([P, SUB, H], f32r, tag="u")
                    nc.sync.dma_start(
                        u_t,
                        sent[b, g * 512 : (g + 1) * 512, :].rearrange(
                            "(q p) h -> p q h", p=P
                        ),
                    )
                    u_tiles.append(u_t)

                em = rowp.tile([1, S], fp32, tag="em")
                z4 = smallp.tile([1, SG], fp32, tag="z4")

                # Software-pipelined over s-groups: transposes of group g are
                # emitted interleaved with the v-matmuls of group g-1 so the
                # PE never runs a long transpose-only stretch (transpose-mode
                # does not keep the HAM clock-gate warm).
                uT_prev = None
                g_prev = None
                for g in range(SG):
                    uT_t = utp.tile([P, KC, 512], f32r, tag="uT")
                    for i in range(KC):
                        tr_ps = ps_tr.tile([P, 512], f32r, tag="tr")
                        for q in range(SUB):
                            nc.tensor.transpose(
                                tr_ps[:, ts(q, P)],
                                u_tiles[g][:, q, ts(i, P)],
                                ident,
                            )
                        nc.vector.tensor_copy(uT_t[:, i, :], tr_ps)
                        if uT_prev is not None:
                            # interleave one k-chunk of prev group's v-matmul
                            vt = ps_vt.tile([P, 512], fp32, tag="vt")
                            for l in range(KC):
                                nc.tensor.matmul(
                                    vt,
                                    lhsT=whT_sb[:, l, ts(i, P)],
                                    rhs=uT_prev[:, l, :],
                                    start=(l == 0),
                                    stop=(l == KC - 1),
                                )
                            mix_t = mixp.tile([P, 512], f32r, tag="mix")
                            nc.scalar.activation(
                                mix_t, vt, Tanh,
                                bias=cT_sb[:, ds(i * BL + b, 1)], scale=1.0,
                            )
                            mix_tiles_prev.append(mix_t)
                    if uT_prev is not None:
                        # alpha-dot + exp + mask for prev group
                        wp = ps_sm.tile([1, 512], fp32, tag="sm")
                        for i in range(KC):
                            nc.tensor.matmul(
                                wp,
                                lhsT=alphaT_sb[:, i : i + 1],
                                rhs=mix_tiles_prev[i],
                                start=(i == 0),
                                stop=(i == KC - 1),
                            )
                        nc.scalar.activation(
                            em[0:1, ts(g_prev, 512)], wp,
                            Exp, bias=ab_sb[0:1, 0:1], scale=1.0,
                        )
                        nc.vector.tensor_tensor(
                            em[0:1, ts(g_prev, 512)],
                            em[0:1, ts(g_prev, 512)],
                            mrow[0:1, ts(g_prev, 512)],
                            MUL,
                        )
                        nc.vector.reduce_sum(
                            z4[0:1, g_prev : g_prev + 1],
                            em[0:1, ts(g_prev, 512)],
                            axis=AX,
                        )
                    uT_prev = uT_t
                    g_prev = g
                    mix_tiles_prev = []

                # drain: last group's compute
                for i in range(KC):
                    vt = ps_vt.tile([P, 512], fp32, tag="vt")
                    for l in range(KC):
                        nc.tensor.matmul(
                            vt,
                            lhsT=whT_sb[:, l, ts(i, P)],
                            rhs=uT_prev[:, l, :],
                            start=(l == 0),
                            stop=(l == KC - 1),
                        )
                    mix_t = mixp.tile([P, 512], f32r, tag="mix")
                    nc.scalar.activation(
                        mix_t, vt, Tanh,
                        bias=cT_sb[:, ds(i * BL + b, 1)], scale=1.0,
                    )
                    mix_tiles_prev.append(mix_t)
                wp = ps_sm.tile([1, 512], fp32, tag="sm")
                for i in range(KC):
                    nc.tensor.matmul(
                        wp,
                        lhsT=alphaT_sb[:, i : i + 1],
                        rhs=mix_tiles_prev[i],
                        start=(i == 0),
                        stop=(i == KC - 1),
                    )
                nc.scalar.activation(
                    em[0:1, ts(g_prev, 512)], wp, Exp, bias=ab_sb[0:1, 0:1], scale=1.0
                )
                nc.vector.tensor_tensor(
                    em[0:1, ts(g_prev, 512)],
                    em[0:1, ts(g_prev, 512)],
                    mrow[0:1, ts(g_prev, 512)],
                    MUL,
                )
                nc.vector.reduce_sum(
                    z4[0:1, g_prev : g_prev + 1], em[0:1, ts(g_prev, 512)], axis=AX
                )

                z1 = smallp.tile([1, 1], fp32, tag="z1")
                nc.vector.reduce_sum(z1, z4, axis=AX)
                inv = smallp.tile([1, 1], fp32, tag="inv")
                nc.vector.reciprocal(inv, z1)

                # eT[:, j] = em[j*128 : (j+1)*128]^T  (unnormalized)
                eT_ps = ps_sm.tile([P, S // P], fp32, tag="sm")
                for j in range(S // P):
                    nc.tensor.matmul(
                        eT_ps[:, j : j + 1],
                        lhsT=em[0:1, ts(j, P)],
                        rhs=ones_f[0:1, 0:1],
                        start=True,
                        stop=True,
                    )
                eT_sb = smallp.tile([P, S // P], f32r, tag="eT")
                nc.vector.tensor_copy(eT_sb, eT_ps)

                # pooled = e^T @ u  (accumulate over the 16 s-chunks)
                pp = ps_pool.tile([1, H], fp32, tag="pool")
                for j in range(S // P):
                    nc.tensor.matmul(
                        pp,
                        lhsT=eT_sb[:, j : j + 1],
                        rhs=u_tiles[j // SUB][:, j % SUB, :],
                        start=(j == 0),
                        stop=(j == S // P - 1),
                    )
                pooled_sb = smallp.tile([1, H], fp32, tag="pooled")
                nc.vector.tensor_scalar_mul(pooled_sb, pp, inv[0:1, 0:1])

                # weight_ output = em / z
                nc.vector.tensor_scalar_mul(em, em, inv[0:1, 0:1])
                nc.sync.dma_start(wout[b : b + 1, :], em)

                # pooledT[:, i] = pooled[i*128 : (i+1)*128]^T
                pT_ps = ps_sm.tile([P, KC], fp32, tag="sm")
                for i in range(KC):
                    nc.tensor.matmul(
                        pT_ps[:, i : i + 1],
                        lhsT=pooled_sb[0:1, ts(i, P)],
                        rhs=ones_f[0:1, 0:1],
                        start=True,
                        stop=True,
                    )
                pT_sb = smallp.tile([P, KC], f32r, tag="pT")
                nc.vector.tensor_copy(pT_sb, pT_ps)

                # att = pooled @ Wh^T + Wh_b
                at_ps = ps_sm.tile([1, H], fp32, tag="sm")
                for i in range(KC):
                    nc.tensor.matmul(
                        at_ps,
                        lhsT=pT_sb[:, i : i + 1],
                        rhs=whT_sb[:, i, :],
                        start=(i == 0),
                        stop=False,
                    )
                nc.tensor.matmul(
                    at_ps, lhsT=ones_r[0:1, 0:1], rhs=whb_sb, start=False, stop=True
                )
                att_sb = smallp.tile([1, H], fp32, tag="att")
                nc.scalar.copy(att_sb, at_ps)
                nc.sync.dma_start(att[b : b + 1, :], att_sb)

    nc.compile()
    return nc


def _get_nc():
    global _NC_CACHE
    if _NC_CACHE is None:
        _NC_CACHE = build_nc()
    return _NC_CACHE


def make_in_maps(inputs):
    f32 = np.float32

    def c(x, dt=f32):
        return np.ascontiguousarray(np.asarray(x, dtype=dt))

    sent_h = c(inputs["sent_h"])
    rel = c(inputs["rel"])
    pool = c(inputs["pool"])
    mask = c(inputs["mask"], np.int32)
    shared = {
        "wgT": c(np.asarray(inputs["Wg_w"], f32).T),
        "whT": c(np.asarray(inputs["Wh_w"], f32).T),
        "wrT": c(np.asarray(inputs["Wr_w"], f32).T),
        "alphaT": c(np.asarray(inputs["alpha_w"], f32).reshape(H)),
        "wgb": c(np.asarray(inputs["Wg_b"], f32).reshape(1, H)),
        "whb": c(np.asarray(inputs["Wh_b"], f32).reshape(1, H)),
        "wrb": c(np.asarray(inputs["Wr_b"], f32).reshape(1, H)),
        "ab": c(np.asarray(inputs["alpha_b"], f32).reshape(1, 1)),
        "idin": np.eye(P, dtype=f32),
        "onein": np.ones((1, P), dtype=f32),
    }
    in_maps = []
    for core in range(NCORES):
        sl = slice(core * BL, (core + 1) * BL)
        m = dict(shared)
        m["sent"] = sent_h[sl]
        m["mask"] = mask[sl]
        m["relT"] = c(rel[sl].T)
        m["poolT"] = c(pool[sl].T)
        in_maps.append(m)
    return in_maps


def run(inputs, trace=False):
    from concourse.bass_utils import run_bass_kernel_spmd

    nc = _get_nc()
    in_maps = make_in_maps(inputs)
    res = run_bass_kernel_spmd(nc, in_maps, core_ids=list(range(NCORES)), trace=trace)
    att = np.concatenate([r["att"] for r in res.results], axis=0).astype(np.float32)
    wei = np.concatenate([r["wout"] for r in res.results], axis=0).astype(np.float32)
    return (att, wei), res


def kernel(**inputs):
    out, _ = run(inputs, trace=False)
    return out


if __name__ == "__main__":
    nc = build_nc()
    print("built ok")


# revision 15
# speedup vs baseline: 1.0012x; 1.0012x over previous
"""Trainium2 Bass kernel for AttentionNet pooling.

Computation (per batch b):
    c[b,:]   = rel[b] @ Wr^T + pool[b] @ Wg^T + Wr_b + Wg_b + Wh_b          [H]
    v[s,k]   = sent_h[b,s,:] @ Wh^T[:,k]                                     [S,H]
    w[s]     = alpha . tanh(c + v[s,:]) + alpha_b                            [S]
    e[s]     = exp(w[s]) * mask[b,s]          (exp without max-sub; masking
                                               multiplicatively after exp)
    weight_  = e / sum(e)
    pooled   = sum_s e[s] * sent_h[b,s,:] / sum(e)                           [H]
    att_res  = pooled @ Wh^T + Wh_b     (since sum(weight_) == 1)            [H]

Sharding: data-parallel over batch B=64 across 8 cores (8 batches/core),
weights replicated.  sent_h is read exactly once from HBM (32 MB/core).

Matmul operands are typed float32r (same fp32 bytes, PE streams 1 col/cycle
instead of 4 for plain fp32, TF32-class precision).  The kernel is one flat
software pipeline over all (batch, s-group) steps: the PE-mode transposes of
step t are interleaved with the v-matmuls of step t-1 (PE transpose-mode
does not register as activity for the HAM clock gate, so long transpose-only
stretches would drop the PE to half clock).
"""

import sys

if "/opt/trn_rl_repo" not in sys.path:
    sys.path.insert(0, "/opt/trn_rl_repo")

import numpy as np

B, S, H, R = 64, 2048, 512, 256
NCORES = 8
BL = B // NCORES  # batches per core
P = 128
KC = H // P      # 4 chunks of the H (contraction / output) dim
SG = S // 512    # 4 s-groups of 512
SUB = 512 // P   # 4 s-subblocks of 128 inside a group

_NC_CACHE = None


def build_nc():
    """Build (once) the single-core Bass/Tile program run SPMD on all cores."""
    import concourse.mybir as mybir
    import concourse.tile as tile
    from concourse import bacc
    from concourse.bass import ts, ds

    fp32 = mybir.dt.float32
    f32r = mybir.dt.float32r
    i32 = mybir.dt.int32
    Tanh = mybir.ActivationFunctionType.Tanh
    Exp = mybir.ActivationFunctionType.Exp
    AX = mybir.AxisListType.X
    MUL = mybir.AluOpType.mult

    nc = bacc.Bacc("TRN2", target_bir_lowering=False, debug=False)

    sent = nc.dram_tensor("sent", [BL, S, H], f32r, kind="ExternalInput")
    mask = nc.dram_tensor("mask", [BL, S], i32, kind="ExternalInput")
    relT = nc.dram_tensor("relT", [R, BL], f32r, kind="ExternalInput")
    poolT = nc.dram_tensor("poolT", [H, BL], f32r, kind="ExternalInput")
    wgT = nc.dram_tensor("wgT", [H, H], f32r, kind="ExternalInput")
    whT = nc.dram_tensor("whT", [H, H], f32r, kind="ExternalInput")
    wrT = nc.dram_tensor("wrT", [R, H], f32r, kind="ExternalInput")
    alphaT = nc.dram_tensor("alphaT", [H], f32r, kind="ExternalInput")
    wgb = nc.dram_tensor("wgb", [1, H], f32r, kind="ExternalInput")
    whb = nc.dram_tensor("whb", [1, H], f32r, kind="ExternalInput")
    wrb = nc.dram_tensor("wrb", [1, H], f32r, kind="ExternalInput")
    ab = nc.dram_tensor("ab", [1, 1], fp32, kind="ExternalInput")
    idin = nc.dram_tensor("idin", [P, P], f32r, kind="ExternalInput")
    onein = nc.dram_tensor("onein", [1, P], f32r, kind="ExternalInput")
    att = nc.dram_tensor("att", [BL, H], fp32, kind="ExternalOutput")
    wout = nc.dram_tensor("wout", [BL, S], fp32, kind="ExternalOutput")

    with tile.TileContext(nc) as tc:
        with (
            tc.tile_pool(name="const", bufs=1) as constp,
            tc.tile_pool(name="u", bufs=10) as up,
            tc.tile_pool(name="uT", bufs=4) as utp,
            tc.tile_pool(name="mix", bufs=6) as mixp,
            tc.tile_pool(name="row", bufs=2) as rowp,
            tc.tile_pool(name="small", bufs=2) as smallp,
            tc.tile_pool(name="ps_tr", bufs=2, space="PSUM") as ps_tr,
            tc.tile_pool(name="ps_vt", bufs=2, space="PSUM") as ps_vt,
            tc.tile_pool(name="ps_sm", bufs=2, space="PSUM") as ps_sm,
            tc.tile_pool(name="ps_pool", bufs=2, space="PSUM") as ps_pool,
        ):
            # ---------------- constants / weights ----------------
            ident = constp.tile([P, P], f32r, tag="ident")
            nc.sync.dma_start(ident, idin[:])
            ones_r = constp.tile([1, P], f32r, tag="ones_r")
            nc.sync.dma_start(ones_r, onein[:])
            ones_f = constp.tile([1, 1], fp32, tag="ones_f")
            nc.vector.memset(ones_f, 1.0)

            whT_sb = constp.tile([P, KC, H], f32r, tag="whT")
            nc.sync.dma_start(whT_sb, whT[:].rearrange("(o p) k -> p o k", p=P))
            wgT_sb = constp.tile([P, KC, H], f32r, tag="wgT")
            nc.sync.dma_start(wgT_sb, wgT[:].rearrange("(o p) k -> p o k", p=P))
            wrT_sb = constp.tile([P, R // P, H], f32r, tag="wrT")
            nc.sync.dma_start(wrT_sb, wrT[:].rearrange("(o p) k -> p o k", p=P))
            alphaT_sb = constp.tile([P, KC], f32r, tag="alphaT")
            nc.sync.dma_start(alphaT_sb, alphaT[:].rearrange("(o p) -> p o", p=P))
            relT_sb = constp.tile([P, R // P, BL], f32r, tag="relT")
            nc.sync.dma_start(relT_sb, relT[:].rearrange("(o p) b -> p o b", p=P))
            poolT_sb = constp.tile([P, KC, BL], f32r, tag="poolT")
            nc.sync.dma_start(poolT_sb, poolT[:].rearrange("(o p) b -> p o b", p=P))
            whb_sb = constp.tile([1, H], f32r, tag="whb")
            nc.sync.dma_start(whb_sb, whb[:])
            wgb_sb = constp.tile([1, H], f32r, tag="wgb")
            nc.sync.dma_start(wgb_sb, wgb[:])
            wrb_sb = constp.tile([1, H], f32r, tag="wrb")
            nc.sync.dma_start(wrb_sb, wrb[:])
            ab_sb = constp.tile([1, 1], fp32, tag="ab")
            nc.sync.dma_start(ab_sb, ab[:])

            # ---------------- c = relation + global_sen + biases ----------------
            c_ps = ps_sm.tile([BL, H], fp32, tag="sm")
            nc.tensor.matmul(c_ps, lhsT=relT_sb[:, 0, :], rhs=wrT_sb[:, 0, :],
                             start=True, stop=False)
            nc.tensor.matmul(c_ps, lhsT=relT_sb[:, 1, :], rhs=wrT_sb[:, 1, :],
                             start=False, stop=False)
            for i in range(KC):
                nc.tensor.matmul(c_ps, lhsT=poolT_sb[:, i, :], rhs=wgT_sb[:, i, :],
                                 start=False, stop=False)
            nc.tensor.matmul(c_ps, lhsT=ones_r[0:1, :BL], rhs=wrb_sb,
                             start=False, stop=False)
            nc.tensor.matmul(c_ps, lhsT=ones_r[0:1, :BL], rhs=wgb_sb,
                             start=False, stop=False)
            nc.tensor.matmul(c_ps, lhsT=ones_r[0:1, :BL], rhs=whb_sb,
                             start=False, stop=True)
            c_sb = smallp.tile([BL, H], f32r, tag="c")
            nc.scalar.copy(c_sb, c_ps)

            # cT[:, i*BL + b] = c[b, i*128 : (i+1)*128]
            cT_ps = ps_sm.tile([P, KC * BL], fp32, tag="sm")
            for i in range(KC):
                nc.tensor.matmul(cT_ps[:, ts(i, BL)], lhsT=c_sb[:, ts(i, P)],
                                 rhs=ident[0:BL, 0:BL], start=True, stop=True)
            cT_sb = constp.tile([P, KC * BL], fp32, tag="cT")
            nc.scalar.copy(cT_sb, cT_ps)

            # ---------------- flat (batch, s-group) pipeline ----------------
            # per-batch live state
            u_tiles = {}   # b -> [4 tiles]
            mrow_t = {}    # b -> [1, S]
            em_t = {}      # b -> [1, S]
            z4_t = {}      # b -> [1, SG]

            def load_batch(b):
                mrow_t[b] = rowp.tile([1, S], fp32, tag="mrow", name=f"mrow{b}")
                nc.gpsimd.dma_start(mrow_t[b], mask[b : b + 1, :])
                tl = []
                for g in range(SG):
                    u_t = up.tile([P, SUB, H], f32r, tag="u")
                    nc.sync.dma_start(
                        u_t,
                        sent[b, g * 512 : (g + 1) * 512, :].rearrange(
                            "(q p) h -> p q h", p=P
                        ),
                    )
                    tl.append(u_t)
                u_tiles[b] = tl

            def emit_group_compute(b, g, uT_t, i):
                """One k-chunk of the v-matmul + tanh for (b, g)."""
                vt = ps_vt.tile([P, 512], fp32, tag="vt")
                for l in range(KC):
                    nc.tensor.matmul(
                        vt,
                        lhsT=whT_sb[:, l, ts(i, P)],
                        rhs=uT_t[:, l, :],
                        start=(l == 0),
                        stop=(l == KC - 1),
                    )
                mix_t = mixp.tile([P, 512], f32r, tag="mix")
                nc.scalar.activation(
                    mix_t, vt, Tanh, bias=cT_sb[:, ds(i * BL + b, 1)], scale=1.0
                )
                return mix_t

            def emit_group_finish(b, g, mix_tiles):
                """alpha-dot + exp + mask + partial sum for (b, g)."""
                wp = ps_sm.tile([1, 512], fp32, tag="sm")
                for i in range(KC):
                    nc.tensor.matmul(
                        wp,
                        lhsT=alphaT_sb[:, i : i + 1],
                        rhs=mix_tiles[i],
                        start=(i == 0),
                        stop=(i == KC - 1),
                    )
                em = em_t[b]
                nc.scalar.activation(
                    em[0:1, ts(g, 512)], wp, Exp, bias=ab_sb[0:1, 0:1], scale=1.0
                )
                nc.vector.tensor_tensor(
                    em[0:1, ts(g, 512)],
                    em[0:1, ts(g, 512)],
                    mrow_t[b][0:1, ts(g, 512)],
                    MUL,
                )
                nc.vector.reduce_sum(
                    z4_t[b][0:1, g : g + 1], em[0:1, ts(g, 512)], axis=AX
                )

            def emit_batch_tail(b):
                """softmax normalize + pooling + att for batch b."""
                em = em_t[b]
                z1 = smallp.tile([1, 1], fp32, tag="z1")
                nc.vector.reduce_sum(z1, z4_t[b], axis=AX)
                inv = smallp.tile([1, 1], fp32, tag="inv")
                nc.vector.reciprocal(inv, z1)

                eT_ps = ps_sm.tile([P, S // P], fp32, tag="sm")
                for j in range(S // P):
                    nc.tensor.matmul(
                        eT_ps[:, j : j + 1],
                        lhsT=em[0:1, ts(j, P)],
                        rhs=ones_f[0:1, 0:1],
                        start=True,
                        stop=True,
                    )
                eT_sb = smallp.tile([P, S // P], f32r, tag="eT")
                nc.vector.tensor_copy(eT_sb, eT_ps)

                pp = ps_pool.tile([1, H], fp32, tag="pool")
                for j in range(S // P):
                    nc.tensor.matmul(
                        pp,
                        lhsT=eT_sb[:, j : j + 1],
                        rhs=u_tiles[b][j // SUB][:, j % SUB, :],
                        start=(j == 0),
                        stop=(j == S // P - 1),
                    )
                pooled_sb = smallp.tile([1, H], fp32, tag="pooled")
                nc.vector.tensor_scalar_mul(pooled_sb, pp, inv[0:1, 0:1])

                # weight_ output = em / z
                nc.vector.tensor_scalar_mul(em, em, inv[0:1, 0:1])
                nc.sync.dma_start(wout[b : b + 1, :], em)

                pT_ps = ps_sm.tile([P, KC], fp32, tag="sm")
                for i in range(KC):
                    nc.tensor.matmul(
                        pT_ps[:, i : i + 1],
                        lhsT=pooled_sb[0:1, ts(i, P)],
                        rhs=ones_f[0:1, 0:1],
                        start=True,
                        stop=True,
                    )
                pT_sb = smallp.tile([P, KC], f32r, tag="pT")
                nc.vector.tensor_copy(pT_sb, pT_ps)

                at_ps = ps_sm.tile([1, H], fp32, tag="sm")
                for i in range(KC):
                    nc.tensor.matmul(
                        at_ps,
                        lhsT=pT_sb[:, i : i + 1],
                        rhs=whT_sb[:, i, :],
                        start=(i == 0),
                        stop=False,
                    )
                nc.tensor.matmul(
                    at_ps, lhsT=ones_r[0:1, 0:1], rhs=whb_sb, start=False, stop=True
                )
                att_sb = smallp.tile([1, H], fp32, tag="att")
                nc.scalar.copy(att_sb, at_ps)
                nc.sync.dma_start(att[b : b + 1, :], att_sb)
                # release per-batch dict entries
                del u_tiles[b], mrow_t[b], em_t[b], z4_t[b]

            steps = [(b, g) for b in range(BL) for g in range(SG)]
            load_batch(0)
            em_t[0] = rowp.tile([1, S], fp32, tag="em", name="em0")
            z4_t[0] = smallp.tile([1, SG], fp32, tag="z4", name="z40")

            prev = None  # (b, g, uT_t)
            for b, g in steps:
                # prefetch next batch's tiles one step before the seam
                if g == SG - 2 and b + 1 < BL:
                    load_batch(b + 1)
                    em_t[b + 1] = rowp.tile([1, S], fp32, tag="em", name=f"em{b+1}")
                    z4_t[b + 1] = smallp.tile([1, SG], fp32, tag="z4", name=f"z4{b+1}")

                uT_t = utp.tile([P, KC, 512], f32r, tag="uT")
                mix_prev = []
                for i in range(KC):
                    tr_ps = ps_tr.tile([P, 512], f32r, tag="tr")
                    for q in range(SUB):
                        nc.tensor.transpose(
                            tr_ps[:, ts(q, P)],
                            u_tiles[b][g][:, q, ts(i, P)],
                            ident,
                        )
                    nc.vector.tensor_copy(uT_t[:, i, :], tr_ps)
                    if prev is not None:
                        mix_prev.append(
                            emit_group_compute(prev[0], prev[1], prev[2], i)
                        )
                if prev is not None:
                    emit_group_finish(prev[0], prev[1], mix_prev)
                    if prev[1] == SG - 1:
                        emit_batch_tail(prev[0])
                prev = (b, g, uT_t)

            # epilogue: drain the final group and batch
            b, g, uT_t = prev
            mix_prev = [emit_group_compute(b, g, uT_t, i) for i in range(KC)]
            emit_group_finish(b, g, mix_prev)
            emit_batch_tail(b)

    nc.compile()
    return nc


def _get_nc():
    global _NC_CACHE
    if _NC_CACHE is None:
        _NC_CACHE = build_nc()
    return _NC_CACHE


def make_in_maps(inputs):
    f32 = np.float32

    def c(x, dt=f32):
        return np.ascontiguousarray(np.asarray(x, dtype=dt))

    sent_h = c(inputs["sent_h"])
    rel = c(inputs["rel"])
    pool = c(inputs["pool"])
    mask = c(inputs["mask"], np.int32)
    shared = {
        "wgT": c(np.asarray(inputs["Wg_w"], f32).T),
        "whT": c(np.asarray(inputs["Wh_w"], f32).T),
        "wrT": c(np.asarray(inputs["Wr_w"], f32).T),
        "alphaT": c(np.asarray(inputs["alpha_w"], f32).reshape(H)),
        "wgb": c(np.asarray(inputs["Wg_b"], f32).reshape(1, H)),
        "whb": c(np.asarray(inputs["Wh_b"], f32).reshape(1, H)),
        "wrb": c(np.asarray(inputs["Wr_b"], f32).reshape(1, H)),
        "ab": c(np.asarray(inputs["alpha_b"], f32).reshape(1, 1)),
        "idin": np.eye(P, dtype=f32),
        "onein": np.ones((1, P), dtype=f32),
    }
    in_maps = []
    for core in range(NCORES):
        sl = slice(core * BL, (core + 1) * BL)
        m = dict(shared)
        m["sent"] = sent_h[sl]
        m["mask"] = mask[sl]
        m["relT"] = c(rel[sl].T)
        m["poolT"] = c(pool[sl].T)
        in_maps.append(m)
    return in_maps


def run(inputs, trace=False):
    from concourse.bass_utils import run_bass_kernel_spmd

    nc = _get_nc()
    in_maps = make_in_maps(inputs)
    res = run_bass_kernel_spmd(nc, in_maps, core_ids=list(range(NCORES)), trace=trace)
    att = np.concatenate([r["att"] for r in res.results], axis=0).astype(np.float32)
    wei = np.concatenate([r["wout"] for r in res.results], axis=0).astype(np.float32)
    return (att, wei), res


def kernel(**inputs):
    out, _ = run(inputs, trace=False)
    return out


if __name__ == "__main__":
    nc = build_nc()
    print("built ok")


# revision 16
# speedup vs baseline: 1.0850x; 1.0836x over previous
"""Trainium2 Bass kernel for AttentionNet pooling.

Computation (per batch b):
    c[b,:]   = rel[b] @ Wr^T + pool[b] @ Wg^T + Wr_b + Wg_b + Wh_b          [H]
    v[s,k]   = sent_h[b,s,:] @ Wh^T[:,k]                                     [S,H]
    w[s]     = alpha . tanh(c + v[s,:]) + alpha_b                            [S]
    e[s]     = exp(w[s]) * mask[b,s]          (exp without max-sub; masking
                                               multiplicatively after exp)
    weight_  = e / sum(e)
    pooled   = sum_s e[s] * sent_h[b,s,:] / sum(e)                           [H]
    att_res  = pooled @ Wh^T + Wh_b     (since sum(weight_) == 1)            [H]

Sharding: data-parallel over batch B=64 across 8 cores (8 batches/core),
weights replicated.  sent_h is read exactly once from HBM (32 MB/core).

Matmul operands are typed float32r (same fp32 bytes, PE streams 1 col/cycle
instead of 4 for plain fp32, TF32-class precision).  The kernel is one flat
software pipeline over all (batch, s-group) steps: the PE-mode transposes of
step t are interleaved with the v-matmuls of step t-1 (PE transpose-mode
does not register as activity for the HAM clock gate, so long transpose-only
stretches would drop the PE to half clock).
"""

import sys

if "/opt/trn_rl_repo" not in sys.path:
    sys.path.insert(0, "/opt/trn_rl_repo")

import numpy as np

B, S, H, R = 64, 2048, 512, 256
NCORES = 8
BL = B // NCORES  # batches per core
P = 128
KC = H // P      # 4 chunks of the H (contraction / output) dim
SG = S // 512    # 4 s-groups of 512
SUB = 512 // P   # 4 s-subblocks of 128 inside a group

_NC_CACHE = None


def build_nc():
    """Build (once) the single-core Bass/Tile program run SPMD on all cores."""
    import concourse.mybir as mybir
    import concourse.tile as tile
    from concourse import bacc
    from concourse.bass import ts, ds

    fp32 = mybir.dt.float32
    f32r = mybir.dt.float32r
    i32 = mybir.dt.int32
    Tanh = mybir.ActivationFunctionType.Tanh
    Exp = mybir.ActivationFunctionType.Exp
    AX = mybir.AxisListType.X
    MUL = mybir.AluOpType.mult

    nc = bacc.Bacc("TRN2", target_bir_lowering=False, debug=False)

    sent = nc.dram_tensor("sent", [BL, S, H], f32r, kind="ExternalInput")
    mask = nc.dram_tensor("mask", [BL, S], i32, kind="ExternalInput")
    relT = nc.dram_tensor("relT", [R, BL], f32r, kind="ExternalInput")
    poolT = nc.dram_tensor("poolT", [H, BL], f32r, kind="ExternalInput")
    wgT = nc.dram_tensor("wgT", [H, H], f32r, kind="ExternalInput")
    whT = nc.dram_tensor("whT", [H, H], f32r, kind="ExternalInput")
    wrT = nc.dram_tensor("wrT", [R, H], f32r, kind="ExternalInput")
    alphaT = nc.dram_tensor("alphaT", [H], f32r, kind="ExternalInput")
    wgb = nc.dram_tensor("wgb", [1, H], f32r, kind="ExternalInput")
    whb = nc.dram_tensor("whb", [1, H], f32r, kind="ExternalInput")
    wrb = nc.dram_tensor("wrb", [1, H], f32r, kind="ExternalInput")
    ab = nc.dram_tensor("ab", [1, 1], fp32, kind="ExternalInput")
    idin = nc.dram_tensor("idin", [P, P], f32r, kind="ExternalInput")
    onein = nc.dram_tensor("onein", [1, P], f32r, kind="ExternalInput")
    att = nc.dram_tensor("att", [BL, H], fp32, kind="ExternalOutput")
    wout = nc.dram_tensor("wout", [BL, S], fp32, kind="ExternalOutput")

    with tile.TileContext(nc) as tc:
        with (
            tc.tile_pool(name="const", bufs=1) as constp,
            tc.tile_pool(name="u", bufs=10) as up,
            tc.tile_pool(name="uT", bufs=4) as utp,
            tc.tile_pool(name="mix", bufs=6) as mixp,
            tc.tile_pool(name="row", bufs=2) as rowp,
            tc.tile_pool(name="small", bufs=2) as smallp,
            tc.tile_pool(name="ps_tr", bufs=2, space="PSUM") as ps_tr,
            tc.tile_pool(name="ps_vt", bufs=2, space="PSUM") as ps_vt,
            tc.tile_pool(name="ps_sm", bufs=2, space="PSUM") as ps_sm,
            tc.tile_pool(name="ps_pool", bufs=2, space="PSUM") as ps_pool,
        ):
            # ---------------- constants / weights ----------------
            ident = constp.tile([P, P], f32r, tag="ident")
            nc.sync.dma_start(ident, idin[:])
            ones_r = constp.tile([1, P], f32r, tag="ones_r")
            nc.sync.dma_start(ones_r, onein[:])
            ones_f = constp.tile([1, 1], fp32, tag="ones_f")
            nc.vector.memset(ones_f, 1.0)

            whT_sb = constp.tile([P, KC, H], f32r, tag="whT")
            nc.sync.dma_start(whT_sb, whT[:].rearrange("(o p) k -> p o k", p=P))
            wgT_sb = constp.tile([P, KC, H], f32r, tag="wgT")
            nc.sync.dma_start(wgT_sb, wgT[:].rearrange("(o p) k -> p o k", p=P))
            wrT_sb = constp.tile([P, R // P, H], f32r, tag="wrT")
            nc.sync.dma_start(wrT_sb, wrT[:].rearrange("(o p) k -> p o k", p=P))
            alphaT_sb = constp.tile([P, KC], f32r, tag="alphaT")
            nc.sync.dma_start(alphaT_sb, alphaT[:].rearrange("(o p) -> p o", p=P))
            relT_sb = constp.tile([P, R // P, BL], f32r, tag="relT")
            nc.sync.dma_start(relT_sb, relT[:].rearrange("(o p) b -> p o b", p=P))
            poolT_sb = constp.tile([P, KC, BL], f32r, tag="poolT")
            nc.sync.dma_start(poolT_sb, poolT[:].rearrange("(o p) b -> p o b", p=P))
            whb_sb = constp.tile([1, H], f32r, tag="whb")
            nc.sync.dma_start(whb_sb, whb[:])
            wgb_sb = constp.tile([1, H], f32r, tag="wgb")
            nc.sync.dma_start(wgb_sb, wgb[:])
            wrb_sb = constp.tile([1, H], f32r, tag="wrb")
            nc.sync.dma_start(wrb_sb, wrb[:])
            ab_sb = constp.tile([1, 1], fp32, tag="ab")
            nc.sync.dma_start(ab_sb, ab[:])

            # ---------------- c = relation + global_sen + biases ----------------
            c_ps = ps_sm.tile([BL, H], fp32, tag="sm")
            nc.tensor.matmul(c_ps, lhsT=relT_sb[:, 0, :], rhs=wrT_sb[:, 0, :],
                             start=True, stop=False)
            nc.tensor.matmul(c_ps, lhsT=relT_sb[:, 1, :], rhs=wrT_sb[:, 1, :],
                             start=False, stop=False)
            for i in range(KC):
                nc.tensor.matmul(c_ps, lhsT=poolT_sb[:, i, :], rhs=wgT_sb[:, i, :],
                                 start=False, stop=False)
            nc.tensor.matmul(c_ps, lhsT=ones_r[0:1, :BL], rhs=wrb_sb,
                             start=False, stop=False)
            nc.tensor.matmul(c_ps, lhsT=ones_r[0:1, :BL], rhs=wgb_sb,
                             start=False, stop=False)
            nc.tensor.matmul(c_ps, lhsT=ones_r[0:1, :BL], rhs=whb_sb,
                             start=False, stop=True)
            c_sb = smallp.tile([BL, H], f32r, tag="c")
            nc.scalar.copy(c_sb, c_ps)

            # cT[:, i*BL + b] = c[b, i*128 : (i+1)*128]
            cT_ps = ps_sm.tile([P, KC * BL], fp32, tag="sm")
            for i in range(KC):
                nc.tensor.matmul(cT_ps[:, ts(i, BL)], lhsT=c_sb[:, ts(i, P)],
                                 rhs=ident[0:BL, 0:BL], start=True, stop=True)
            cT_sb = constp.tile([P, KC * BL], fp32, tag="cT")
            nc.scalar.copy(cT_sb, cT_ps)

            # ---------------- flat (batch, s-group) pipeline ----------------
            # per-batch live state
            u_tiles = {}   # b -> [4 tiles]
            pp_t = {}      # b -> pooled PSUM accumulator
            mrow_t = {}    # b -> [1, S]
            em_t = {}      # b -> [1, S]
            z4_t = {}      # b -> [1, SG]

            def load_batch(b):
                mrow_t[b] = rowp.tile([1, S], fp32, tag="mrow", name=f"mrow{b}")
                nc.gpsimd.dma_start(mrow_t[b], mask[b : b + 1, :])
                tl = []
                for g in range(SG):
                    u_t = up.tile([P, SUB, H], f32r, tag="u")
                    nc.sync.dma_start(
                        u_t,
                        sent[b, g * 512 : (g + 1) * 512, :].rearrange(
                            "(q p) h -> p q h", p=P
                        ),
                    )
                    tl.append(u_t)
                u_tiles[b] = tl

            def emit_group_compute(b, g, uT_t, i):
                """One k-chunk of the v-matmul + tanh for (b, g)."""
                vt = ps_vt.tile([P, 512], fp32, tag="vt")
                for l in range(KC):
                    nc.tensor.matmul(
                        vt,
                        lhsT=whT_sb[:, l, ts(i, P)],
                        rhs=uT_t[:, l, :],
                        start=(l == 0),
                        stop=(l == KC - 1),
                    )
                mix_t = mixp.tile([P, 512], f32r, tag="mix")
                nc.scalar.activation(
                    mix_t, vt, Tanh, bias=cT_sb[:, ds(i * BL + b, 1)], scale=1.0
                )
                return mix_t

            def emit_group_finish(b, g, mix_tiles):
                """alpha-dot + exp + mask + partial sum + pooling for (b, g)."""
                wp = ps_sm.tile([1, 512], fp32, tag="sm")
                for i in range(KC):
                    nc.tensor.matmul(
                        wp,
                        lhsT=alphaT_sb[:, i : i + 1],
                        rhs=mix_tiles[i],
                        start=(i == 0),
                        stop=(i == KC - 1),
                    )
                em = em_t[b]
                nc.scalar.activation(
                    em[0:1, ts(g, 512)], wp, Exp, bias=ab_sb[0:1, 0:1], scale=1.0
                )
                nc.vector.tensor_tensor(
                    em[0:1, ts(g, 512)],
                    em[0:1, ts(g, 512)],
                    mrow_t[b][0:1, ts(g, 512)],
                    MUL,
                )
                nc.vector.reduce_sum(
                    z4_t[b][0:1, g : g + 1], em[0:1, ts(g, 512)], axis=AX
                )
                # row->col flip of this group's masked weights, then its four
                # pooling matmuls accumulate into the batch PSUM right away
                # (keeps these matmuls inside the warm pipeline instead of a
                # long cold batch tail).
                eTg_ps = ps_sm.tile([P, SUB], fp32, tag="sm")
                for q in range(SUB):
                    nc.tensor.matmul(
                        eTg_ps[:, q : q + 1],
                        lhsT=em[0:1, ds(g * 512 + q * P, P)],
                        rhs=ones_f[0:1, 0:1],
                        start=True,
                        stop=True,
                    )
                eTg_sb = smallp.tile([P, SUB], f32r, tag="eTg")
                nc.vector.tensor_copy(eTg_sb, eTg_ps)
                if g == 0:
                    pp_t[b] = ps_pool.tile([1, H], fp32, tag="pool", name=f"pp{b}")
                for q in range(SUB):
                    nc.tensor.matmul(
                        pp_t[b],
                        lhsT=eTg_sb[:, q : q + 1],
                        rhs=u_tiles[b][g][:, q, :],
                        start=(g == 0 and q == 0),
                        stop=(g == SG - 1 and q == SUB - 1),
                    )

            def emit_batch_tail(b):
                """softmax normalize + att for batch b (pooling already done)."""
                em = em_t[b]
                z1 = smallp.tile([1, 1], fp32, tag="z1")
                nc.vector.reduce_sum(z1, z4_t[b], axis=AX)
                inv = smallp.tile([1, 1], fp32, tag="inv")
                nc.vector.reciprocal(inv, z1)

                pooled_sb = smallp.tile([1, H], fp32, tag="pooled")
                nc.vector.tensor_scalar_mul(pooled_sb, pp_t[b], inv[0:1, 0:1])

                # weight_ output = em / z
                nc.vector.tensor_scalar_mul(em, em, inv[0:1, 0:1])
                nc.sync.dma_start(wout[b : b + 1, :], em)

                pT_ps = ps_sm.tile([P, KC], fp32, tag="sm")
                for i in range(KC):
                    nc.tensor.matmul(
                        pT_ps[:, i : i + 1],
                        lhsT=pooled_sb[0:1, ts(i, P)],
                        rhs=ones_f[0:1, 0:1],
                        start=True,
                        stop=True,
                    )
                pT_sb = smallp.tile([P, KC], f32r, tag="pT")
                nc.vector.tensor_copy(pT_sb, pT_ps)

                at_ps = ps_sm.tile([1, H], fp32, tag="sm")
                for i in range(KC):
                    nc.tensor.matmul(
                        at_ps,
                        lhsT=pT_sb[:, i : i + 1],
                        rhs=whT_sb[:, i, :],
                        start=(i == 0),
                        stop=False,
                    )
                nc.tensor.matmul(
                    at_ps, lhsT=ones_r[0:1, 0:1], rhs=whb_sb, start=False, stop=True
                )
                att_sb = smallp.tile([1, H], fp32, tag="att")
                nc.scalar.copy(att_sb, at_ps)
                nc.sync.dma_start(att[b : b + 1, :], att_sb)
                # release per-batch dict entries
                del u_tiles[b], mrow_t[b], em_t[b], z4_t[b], pp_t[b]

            steps = [(b, g) for b in range(BL) for g in range(SG)]
            load_batch(0)
            em_t[0] = rowp.tile([1, S], fp32, tag="em", name="em0")
            z4_t[0] = smallp.tile([1, SG], fp32, tag="z4", name="z40")

            prev = None  # (b, g, uT_t)
            for b, g in steps:
                # prefetch next batch's tiles one step before the seam
                if g == SG - 2 and b + 1 < BL:
                    load_batch(b + 1)
                    em_t[b + 1] = rowp.tile([1, S], fp32, tag="em", name=f"em{b+1}")
                    z4_t[b + 1] = smallp.tile([1, SG], fp32, tag="z4", name=f"z4{b+1}")

                uT_t = utp.tile([P, KC, 512], f32r, tag="uT")
                mix_prev = []
                for i in range(KC):
                    tr_ps = ps_tr.tile([P, 512], f32r, tag="tr")
                    for q in range(SUB):
                        nc.tensor.transpose(
                            tr_ps[:, ts(q, P)],
                            u_tiles[b][g][:, q, ts(i, P)],
                            ident,
                        )
                    nc.vector.tensor_copy(uT_t[:, i, :], tr_ps)
                    if prev is not None:
                        mix_prev.append(
                            emit_group_compute(prev[0], prev[1], prev[2], i)
                        )
                if prev is not None:
                    emit_group_finish(prev[0], prev[1], mix_prev)
                    if prev[1] == SG - 1:
                        emit_batch_tail(prev[0])
                prev = (b, g, uT_t)

            # epilogue: drain the final group and batch
            b, g, uT_t = prev
            mix_prev = [emit_group_compute(b, g, uT_t, i) for i in range(KC)]
            emit_group_finish(b, g, mix_prev)
            emit_batch_tail(b)

    nc.compile()
    return nc


def _get_nc():
    global _NC_CACHE
    if _NC_CACHE is None:
        _NC_CACHE = build_nc()
    return _NC_CACHE


def make_in_maps(inputs):
    f32 = np.float32

    def c(x, dt=f32):
        return np.ascontiguousarray(np.asarray(x, dtype=dt))

    sent_h = c(inputs["sent_h"])
    rel = c(inputs["rel"])
    pool = c(inputs["pool"])
    mask = c(inputs["mask"], np.int32)
    shared = {
        "wgT": c(np.asarray(inputs["Wg_w"], f32).T),
        "whT": c(np.asarray(inputs["Wh_w"], f32).T),
        "wrT": c(np.asarray(inputs["Wr_w"], f32).T),
        "alphaT": c(np.asarray(inputs["alpha_w"], f32).reshape(H)),
        "wgb": c(np.asarray(inputs["Wg_b"], f32).reshape(1, H)),
        "whb": c(np.asarray(inputs["Wh_b"], f32).reshape(1, H)),
        "wrb": c(np.asarray(inputs["Wr_b"], f32).reshape(1, H)),
        "ab": c(np.asarray(inputs["alpha_b"], f32).reshape(1, 1)),
        "idin": np.eye(P, dtype=f32),
        "onein": np.ones((1, P), dtype=f32),
    }
    in_maps = []
    for core in range(NCORES):
        sl = slice(core * BL, (core + 1) * BL)
        m = dict(shared)
        m["sent"] = sent_h[sl]
        m["mask"] = mask[sl]
        m["relT"] = c(rel[sl].T)
        m["poolT"] = c(pool[sl].T)
        in_maps.append(m)
    return in_maps


def run(inputs, trace=False):
    from concourse.bass_utils import run_bass_kernel_spmd

    nc = _get_nc()
    in_maps = make_in_maps(inputs)
    res = run_bass_kernel_spmd(nc, in_maps, core_ids=list(range(NCORES)), trace=trace)
    att = np.concatenate([r["att"] for r in res.results], axis=0).astype(np.float32)
    wei = np.concatenate([r["wout"] for r in res.results], axis=0).astype(np.float32)
    return (att, wei), res


def kernel(**inputs):
    out, _ = run(inputs, trace=False)
    return out


if __name__ == "__main__":
    nc = build_nc()
    print("built ok")
